# revision 4
# baseline (speedup 1.0000x reference)
"""GCN layer kernel for Trainium2, batch-parallel across 8 NeuronCores.

out[b] = D^-1/2 (A[b]+I) D^-1/2 @ x[b] @ W.T + b_vec

Per-core dataflow (core c owns batch element c):
  - adj slice [N,N] is streamed from HBM exactly once in 16 row-tiles,
    each split into two column-half DMAs matching the scalar/vector cast
    split, so each cast engine starts as soon as its half lands.
  - Each fp32 half is cast to fp16 with accum_out row-sum partials
    (degrees come for free with the cast pass).
  - The PE transposes each 128x128 block into a resident A^T buffer in
    SBUF; transpose-PSUM drains alternate vector/scalar.
  - Degree scaling is folded into x (x' = d*x) and the output
    (out = d * (...)), so adj_norm is never materialized.
  - Aggregation accumulates aggT[f, n] += x'_j^T AT_j in PSUM. Chunks
    0-2 use 512-wide moving ranges (ready when their A rows land at
    tiles 3/7/11); the last 512 columns are split into four 128-wide
    blocks so their steps fire at max(u, j) instead of all at tile 15.
    The +I self-loop rides as x'_u^T @ I identity matmuls.
  - x rides the software-DGE (gpsimd) queue as one big load: SWDGE
    descriptor generation is ~15x cheaper per descriptor than HWDGE and
    the gpsimd engine is otherwise idle.
  - Tail: per 128-block, drain -> o2 = aggT^T @ W^T -> d*o2 + b ->
    output writes alternating across both HWDGE queues.
"""

import numpy as np

B, N, F = 8, 2048, 128
P = 128                 # partition tile / block size
NT = N // P             # 16 row tiles
NCHUNK = 512            # moving-dim chunk for early aggregation chunks
NBIG = 3                # chunks 0..2 are 512-wide; the last is 4x128
WARMUP = 40             # dummy PE ops at start to lift the HAM clock gate

_PROGRAM_CACHE = {}


def _build_program(agg_dtype_name="float16", csplit_blocks=9, agg_cap=6):
    import concourse.bacc as bacc
    import concourse.bass as bass
    import concourse.tile as tile
    from concourse import mybir
    from concourse.masks import make_identity

    f32 = mybir.dt.float32
    agg_dt = getattr(mybir.dt, agg_dtype_name)
    csplit = csplit_blocks * P  # scalar casts [0:csplit], vector the rest

    nc = bacc.Bacc(
        "TRN2",
        target_bir_lowering=False,
        debug=False,
        num_devices=B,
        enable_partition_id=False,
    )
    x_d = nc.dram_tensor("x", [N, F], f32, kind="ExternalInput")
    a_d = nc.dram_tensor("adj", [N, N], f32, kind="ExternalInput")
    w_d = nc.dram_tensor("W", [F, F], f32, kind="ExternalInput")
    b_d = nc.dram_tensor("b", [F], f32, kind="ExternalInput")
    o_d = nc.dram_tensor("out", [N, F], f32, kind="ExternalOutput")

    with tile.TileContext(nc) as tc:
        with (
            tc.tile_pool(name="singles", bufs=1) as singles,
            tc.tile_pool(name="a_in", bufs=6) as a_in,
            tc.tile_pool(name="a_cast", bufs=4) as a_cast,
            tc.tile_pool(name="small", bufs=8) as small,
            tc.tile_pool(name="tp_psum", bufs=2, space="PSUM") as tp_psum,
            tc.tile_pool(name="agg_psum", bufs=1, space="PSUM") as agg_psum,
            tc.tile_pool(name="o2_psum", bufs=2, space="PSUM") as o2_psum,
        ):
            ident_t = singles.tile([P, P], agg_dt)   # transpose rhs + self-loop
            make_identity(nc, ident_t)
            ident_f = singles.tile([P, P], f32)      # W transpose rhs
            make_identity(nc, ident_f)

            # dummy PE activity during the DMA fill phase lifts the HAM
            # clock gate (PE only ramps after ~4us of sustained activity)
            for w in range(WARMUP // 8):
                wtp = tp_psum.tile([P, 8, P], agg_dt, name="tp", tag="tp")
                for s in range(8):
                    nc.tensor.transpose(wtp[:, s, :], ident_t, ident_t)

            # W^T: load W [o,f], transpose once -> wt_sb [f,o]
            w_sb = singles.tile([P, P], f32)
            nc.scalar.dma_start(w_sb, w_d[:, :])
            wt_ps = o2_psum.tile([P, P], f32, tag="o2")
            nc.tensor.transpose(wt_ps, w_sb, ident_f)
            wt_sb = singles.tile([P, P], agg_dt)
            nc.scalar.copy(wt_sb, wt_ps)

            # bias broadcast across partitions: b_sb[p, o] = b[o]
            b_sb = singles.tile([P, F], f32)
            b_ap = b_d[:]
            nc.scalar.dma_start(
                b_sb, bass.AP(tensor=b_ap.tensor, offset=b_ap.offset, ap=[[0, P], *b_ap.ap])
            )

            # x: one big SWDGE load (gpsimd) -> x_sb[p, t, f] = x[t*P+p, f]
            x_sb = singles.tile([P, NT, F], f32)
            x_ap = x_d[:, :]
            nc.gpsimd.dma_start(
                x_sb,
                bass.AP(
                    tensor=x_ap.tensor,
                    offset=x_ap.offset,
                    ap=[[F, P], [F * P, NT], [1, F]],
                ),
            )

            xp_sb = singles.tile([P, NT, F], agg_dt)   # x' = d * x
            at_sb = singles.tile([P, NT, N], agg_dt)   # resident A^T
            aggt_sb = singles.tile([P, N], agg_dt)     # aggT = (A+I)x' transposed
            out_sb = singles.tile([P, NT, F], f32)
            d_all = singles.tile([P, NT], f32)         # d = (rowsum+1)^-1/2

            # PSUM accumulators: three 512-wide chunks + one bank of 4x128
            # blocks for the final 512 columns (distinct tags keep all
            # resident; they accumulate across the whole kernel)
            agg_big = [
                agg_psum.tile([P, NCHUNK], f32, name=f"agg_c{i}", tag=f"aggc{i}")
                for i in range(NBIG)
            ]
            agg3 = agg_psum.tile([P, 4, P], f32, name="agg3", tag="agg3")
            # the four 128-col blocks in this bank accumulate as independent
            # interleaved groups; a matmul start=True would zero the whole
            # 2KB zero region (bank), clobbering its neighbours. Pre-zero
            # the bank once and run every agg3 matmul with start=False.
            for ui in range(4):
                nc.vector.tensor_scalar_mul(agg3[:, ui, :], ident_f, 0.0)

            # --- aggregation step plan -------------------------------
            # step kinds:
            #   ("cj", c, j): agg_big[c] += xp_j^T @ AT[:, j, 512c:512c+512]
            #   ("cu", c, u): self-loop on big chunk c at block u
            #   ("uj", u, j): agg3[:, u-12, :] += xp_j^T @ AT[:, j, 128u:...]
            #   ("uu", u):    self-loop on agg3 block u
            steps = []
            for c in range(NBIG):
                ready_c = 4 * c + 3
                for j in range(NT):
                    steps.append((max(ready_c, j), 0, "cj", c, j))
                for u in range(4 * c, 4 * (c + 1)):
                    steps.append((max(ready_c, u), 1, "cu", c, u))
            for u in range(4 * NBIG, NT):
                for j in range(NT):
                    steps.append((max(u, j), 0, "uj", u, j))
                steps.append((u, 1, "uu", u, 0))

            # accumulation-group bookkeeping: first/last per PSUM region.
            # big chunks: whole-chunk j-steps come before sub-range ident
            # steps at equal ready time (sort key), so start=True always
            # covers the full chunk range.
            big_total = [NT + 4] * NBIG
            big_emitted = [0] * NBIG
            u_total = [NT + 1] * 4
            u_emitted = [0] * 4

            def emit_step(kind, a, bopt):
                if kind in ("cj", "cu"):
                    c = a
                    first = big_emitted[c] == 0
                    big_emitted[c] += 1
                    last = big_emitted[c] == big_total[c]
                    if kind == "cj":
                        j = bopt
                        nc.tensor.matmul(
                            agg_big[c],
                            xp_sb[:, j, :],
                            at_sb[:, j, NCHUNK * c : NCHUNK * (c + 1)],
                            start=first,
                            stop=last,
                        )
                    else:
                        u = bopt
                        off = P * (u - 4 * c)
                        nc.tensor.matmul(
                            agg_big[c][:, off : off + P],
                            xp_sb[:, u, :],
                            ident_t,
                            start=first,
                            stop=last,
                        )
                else:
                    u = a
                    ui = u - 4 * NBIG
                    u_emitted[ui] += 1
                    last = u_emitted[ui] == u_total[ui]
                    if kind == "uj":
                        j = bopt
                        nc.tensor.matmul(
                            agg3[:, ui, :],
                            xp_sb[:, j, :],
                            at_sb[:, j, P * u : P * (u + 1)],
                            start=False,
                            stop=last,
                            skip_group_check=True,
                        )
                    else:
                        nc.tensor.matmul(
                            agg3[:, ui, :],
                            xp_sb[:, u, :],
                            ident_t,
                            start=False,
                            stop=last,
                            skip_group_check=True,
                        )

            steps.sort(key=lambda s: (s[0], s[1]))
            step_idx = 0

            def fire_agg_steps(t):
                nonlocal step_idx
                budget = agg_cap if t < NT - 1 else len(steps)
                while budget > 0 and step_idx < len(steps) and steps[step_idx][0] <= t:
                    _, _, kind, a, bopt = steps[step_idx]
                    emit_step(kind, a, bopt)
                    step_idx += 1
                    budget -= 1

            for t in range(NT):
                # two column-half loads matching the cast split
                a_t = a_in.tile([P, N], f32)
                nc.sync.dma_start(a_t[:, :csplit], a_d[P * t : P * (t + 1), :csplit])
                nc.sync.dma_start(a_t[:, csplit:], a_d[P * t : P * (t + 1), csplit:])

                # cast + row-sum partials; each engine waits only on its half
                a_c = a_cast.tile([P, N], agg_dt)
                rs_a = small.tile([P, 1], f32)
                nc.scalar.activation(
                    a_c[:, :csplit],
                    a_t[:, :csplit],
                    mybir.ActivationFunctionType.Copy,
                    accum_out=rs_a,
                )
                rs_b = small.tile([P, 1], f32)
                nc.vector.tensor_scalar(
                    a_c[:, csplit:],
                    a_t[:, csplit:],
                    1.0,
                    None,
                    op0=mybir.AluOpType.mult,
                    op1=mybir.AluOpType.add,  # accum reduce op
                    accum_out=rs_b,
                )

                # d_t = (rs_a + rs_b + 1)^-1/2  (+1 = self loop)
                rs = small.tile([P, 1], f32)
                nc.gpsimd.tensor_add(rs, rs_a, rs_b)
                sq = small.tile([P, 1], f32)
                nc.scalar.activation(
                    sq, rs, mybir.ActivationFunctionType.Sqrt, bias=1.0, scale=1.0
                )
                nc.vector.reciprocal(d_all[:, t : t + 1], sq)

                # x'_t = d_t * x_t (per-partition scale)
                nc.vector.tensor_scalar_mul(
                    xp_sb[:, t, :], x_sb[:, t, :], d_all[:, t : t + 1]
                )

                # transpose blocks on the PE in two groups of 8; drains
                # alternate vector/scalar
                for g in range(2):
                    tp = tp_psum.tile([P, 8, P], agg_dt, name="tp", tag="tp")
                    for s in range(8):
                        j = 8 * g + s
                        nc.tensor.transpose(
                            tp[:, s, :], a_c[:, P * j : P * (j + 1)], ident_t
                        )
                    dst = at_sb[:, 8 * g : 8 * (g + 1), P * t : P * (t + 1)]
                    if g == 0:
                        nc.vector.tensor_copy(dst, tp)
                    else:
                        nc.scalar.copy(dst, tp)

                fire_agg_steps(t)

            # tail: per 128-block drain -> linear -> scale+bias -> write
            def tail_block(u, drain_src, drain_eng):
                dst = aggt_sb[:, P * u : P * (u + 1)]
                drain_eng(dst, drain_src)
                o2 = o2_psum.tile([P, P], f32, name="o2", tag="o2")
                nc.tensor.matmul(o2, dst, wt_sb, start=True, stop=True)
                nc.vector.scalar_tensor_tensor(
                    out_sb[:, u, :],
                    o2,
                    d_all[:, u : u + 1],
                    b_sb,
                    op0=mybir.AluOpType.mult,
                    op1=mybir.AluOpType.add,
                )
                eng = nc.sync if u % 2 == 0 else nc.scalar
                eng.dma_start(o_d[P * u : P * (u + 1), :], out_sb[:, u, :])

            # big chunks close in order (their (c,15) steps fired first in
            # the flush); drain each as one copy, then emit its blocks
            for c in range(NBIG):
                dst = aggt_sb[:, NCHUNK * c : NCHUNK * (c + 1)]
                if c % 2 == 0:
                    nc.vector.tensor_copy(dst, agg_big[c])
                else:
                    nc.scalar.copy(dst, agg_big[c])
                for u in range(4 * c, 4 * (c + 1)):
                    o2 = o2_psum.tile([P, P], f32, name="o2", tag="o2")
                    nc.tensor.matmul(
                        o2, aggt_sb[:, P * u : P * (u + 1)], wt_sb, start=True, stop=True
                    )
                    nc.vector.scalar_tensor_tensor(
                        out_sb[:, u, :],
                        o2,
                        d_all[:, u : u + 1],
                        b_sb,
                        op0=mybir.AluOpType.mult,
                        op1=mybir.AluOpType.add,
                    )
                    eng = nc.sync if u % 2 == 0 else nc.scalar
                    eng.dma_start(o_d[P * u : P * (u + 1), :], out_sb[:, u, :])

            # final four blocks drain individually as each closes
            for ui in range(4):
                u = 4 * NBIG + ui
                drain = nc.vector.tensor_copy if ui % 2 == 0 else nc.scalar.copy
                tail_block(u, agg3[:, ui, :], drain)

    nc.compile()
    return nc


def get_program(agg_dtype_name="float16", csplit_blocks=9, agg_cap=6):
    key = (agg_dtype_name, csplit_blocks, agg_cap)
    if key not in _PROGRAM_CACHE:
        _PROGRAM_CACHE[key] = _build_program(agg_dtype_name, csplit_blocks, agg_cap)
    return _PROGRAM_CACHE[key]


def kernel(x, adj, W, b, _trace=False, _agg_dtype="float16", _csplit=9, _cap=6):
    from concourse.bass_utils import run_bass_kernel_spmd

    nc = get_program(_agg_dtype, _csplit, _cap)
    x = np.ascontiguousarray(np.asarray(x), dtype=np.float32)
    adj = np.ascontiguousarray(np.asarray(adj), dtype=np.float32)
    W = np.ascontiguousarray(np.asarray(W), dtype=np.float32)
    b = np.ascontiguousarray(np.asarray(b), dtype=np.float32)

    in_maps = [
        {"x": x[c], "adj": adj[c], "W": W, "b": b} for c in range(B)
    ]
    res = run_bass_kernel_spmd(
        nc, in_maps, list(range(B)), trace=_trace, trace_cores=[0] if _trace else None
    )
    out = np.stack([res.results[c]["out"] for c in range(B)], axis=0)
    if _trace:
        return out, res
    return out
